# revision 1
# baseline (speedup 1.0000x reference)
"""Trainium2 Bass kernel for nn_ConditionedLM (BiLSTM table encoder -> LSTM LM -> vocab decoder).

Strategy (8 NeuronCores, SPMD — one program, per-core data):
  * Embedding gathers + input projections (x @ Wih.T + b) computed on-device,
    replicated on every core.
  * LSTM recurrences (encoder fwd+bwd fused, then the LM) replicated on all
    cores: per-step cost is dominated by streaming Whh through the PE array
    (batch-independent), and an 8-way shard would need a per-step all-gather
    whose ~5us floor exceeds the whole step.  Replication also means the
    decode needs no communication.
  * Decoder matmul (ys @ Wdec.T + bdec) sharded over vocab: core m computes
    logits[:, :, m*VS:(m+1)*VS] — the memory-heavy part (206MB Wdec read,
    412MB logits write across 8 cores).

Per-step trick: gates computed with 4 PE column-group-tiled matmuls
(tile_position=(0,32j)), one per gate, all four streams concurrent in the
128x128 array (M=B=16 per 32-col strip).  Gate g lands in PSUM partitions
[32g, 32g+16), so one Sigmoid over partitions 0..79 covers (i, f, o) and one
Tanh over [96,112) covers g (weights host-reordered i,f,g,o -> i,f,o,g).
Input projections enter the accumulation as a K=16 matmul with an identity
lhsT.  h is re-transposed each step via 8 PE transposes; for the LM the
transposed h goes straight into the ys.T buffer that decode uses as lhsT.
"""

import numpy as np
import ml_dtypes
from contextlib import ExitStack

import concourse.bass as bass
import concourse.mybir as mybir
import concourse.tile as tile
from concourse import bacc
from concourse.bass_utils import run_bass_kernel_spmd
from concourse.masks import make_identity

dt = mybir.dt
bf16 = ml_dtypes.bfloat16

V, E, He, H = 50257, 512, 512, 1024
B, T, Lt = 16, 128, 64
NCORES = 8
VS = (V + NCORES - 1) // NCORES  # 6283 vocab rows per core (padded)
N_TOK = B * T                    # 2048
N_TAB = B * Lt                   # 1024

MM_DT = dt.bfloat16              # matmul operand dtype (accum stays fp32)
MM_NP = bf16
EW_DT = dt.float32               # elementwise/state dtype

_CACHE = {}
LT_STEPS = Lt
T_STEPS = T
DEC_NC = None
SKIP = set()


def _gate_perm(h):
    # torch gate order i,f,g,o -> i,f,o,g
    return np.concatenate([np.arange(0, h), np.arange(h, 2 * h),
                           np.arange(3 * h, 4 * h), np.arange(2 * h, 3 * h)])


def _ceil_div(a, b):
    return (a + b - 1) // b


def _bcast_ap(dram_tensor, n_free):
    """AP reading dram_tensor's single row broadcast to 128 partitions."""
    return bass.AP(dram_tensor, 0, [[0, 128], [1, n_free]])


def build_bass():
    nc = bacc.Bacc()

    embed_d = nc.dram_tensor("embed", [V, E], dt.float32, kind="ExternalInput")
    tembed_d = nc.dram_tensor("tembed", [V, E], dt.float32, kind="ExternalInput")
    idx_d = nc.dram_tensor("idx_t", [128, 24], dt.int32, kind="ExternalInput")
    wih_enc_d = nc.dram_tensor("wih_enc_t", [E, 8 * He], MM_DT, kind="ExternalInput")
    wih_lm_d = nc.dram_tensor("wih_lm_t", [E, 4 * H], MM_DT, kind="ExternalInput")
    whh_f_d = nc.dram_tensor("whh_f_t", [He, 4 * He], MM_DT, kind="ExternalInput")
    whh_b_d = nc.dram_tensor("whh_b_t", [He, 4 * He], MM_DT, kind="ExternalInput")
    whh_lm_d = nc.dram_tensor("whh_lm_t", [H, 4 * H], MM_DT, kind="ExternalInput")
    b_enc_d = nc.dram_tensor("b_enc", [1, 8 * He], dt.float32, kind="ExternalInput")
    b_lm_d = nc.dram_tensor("b_lm", [1, 4 * H], dt.float32, kind="ExternalInput")
    wdec_d = nc.dram_tensor("wdec_t", [H, VS], MM_DT, kind="ExternalInput")
    bdec_d = nc.dram_tensor("bdec_s", [1, VS], dt.float32, kind="ExternalInput")
    h0_d = nc.dram_tensor("enc_h0", [2, B, He], dt.float32, kind="ExternalInput")
    c0_d = nc.dram_tensor("enc_c0", [2, B, He], dt.float32, kind="ExternalInput")
    out_d = nc.dram_tensor("out", [N_TOK, VS], dt.float32, kind="ExternalOutput")

    with tile.TileContext(nc) as tc, ExitStack() as ctx:
        # DRAM intermediates for input projections (row r = 16t+b)
        dram = ctx.enter_context(tc.tile_pool(name="dram", bufs=1, space="DRAM"))
        xp_lm_d = dram.tile([N_TOK, 4 * H], MM_DT)       # [2048, 4096]
        xp_enc_d = dram.tile([N_TAB, 8 * He], MM_DT)     # [1024, fwd|bwd]

        const = ctx.enter_context(tc.tile_pool(name="const", bufs=1))
        ident = const.tile([128, 128], dt.float32)
        make_identity(nc, ident[:])
        ident_mm = const.tile([16, 16], MM_DT)
        make_identity(nc, ident_mm[:])
        ident_bf = const.tile([128, 128], MM_DT)
        make_identity(nc, ident_bf[:])

        idx_p = ctx.enter_context(tc.tile_pool(name="idx", bufs=1))
        idx_sb = idx_p.tile([128, 24], dt.int32)
        nc.sync.dma_start(idx_sb[:], idx_d[:])

        psum_mm = ctx.enter_context(
            tc.tile_pool(name="psum_mm", bufs=2, space="PSUM"))
        ysT_p = ctx.enter_context(tc.tile_pool(name="ysT", bufs=1))
        ysT = ysT_p.tile([128, 8, N_TOK], MM_DT)

        # =========================================================
        # Phase A+B: gather + transpose embeddings, input projections
        # =========================================================
        with tc.tile_pool(name="gather", bufs=3) as gpool, \
             tc.tile_pool(name="embT", bufs=1) as epool, \
             tc.tile_pool(name="bias", bufs=1) as bpool, \
             tc.tile_pool(name="wih", bufs=5) as wpool, \
             tc.tile_pool(name="xpout", bufs=4) as xpo:
            embT = epool.tile([128, E // 128, N_TOK], MM_DT)
            tembT = epool.tile([128, E // 128, N_TAB], MM_DT)

            for src, n_rows, dst, icol0 in (
                    (embed_d, N_TOK, embT, 0), (tembed_d, N_TAB, tembT, 16)):
                for g in range(n_rows // 128):
                    rows = gpool.tile([128, E], MM_DT, tag="rows")
                    nc.gpsimd.indirect_dma_start(
                        out=rows[:], out_offset=None, in_=src[:],
                        in_offset=bass.IndirectOffsetOnAxis(
                            ap=idx_sb[:, icol0 + g:icol0 + g + 1], axis=0))
                    pt = psum_mm.tile([128, E // 128, 128], MM_DT, tag="mm")
                    for k in range(E // 128):
                        nc.tensor.transpose(pt[:, k, :],
                                            rows[:, 128 * k:128 * (k + 1)],
                                            ident_bf[:])
                    if g % 2 == 0:
                        nc.vector.tensor_copy(dst[:, :, 128 * g:128 * (g + 1)], pt[:])
                    else:
                        nc.scalar.copy(dst[:, :, 128 * g:128 * (g + 1)], pt[:])

            for (bias_src, wih_src, lhsT, n_rows, is_enc) in (
                    (b_enc_d, wih_enc_d, tembT, N_TAB, True),
                    (b_lm_d, wih_lm_d, embT, N_TOK, False)):
                nb = 8
                bias_bc = bpool.tile([128, 4096], dt.float32, tag="bbc")
                nc.sync.dma_start(bias_bc[:], _bcast_ap(bias_src, 4096))
                wchunks = []
                for k in range(4):
                    wc = wpool.tile([128, 4096], MM_DT, tag="wih")
                    nc.sync.dma_start(wc[:], wih_src[128 * k:128 * (k + 1), :])
                    wchunks.append(wc)
                for g in range(n_rows // 128):
                    for n in range(nb):
                        px = psum_mm.tile([128, 512], dt.float32, tag="mm")
                        for k in range(4):
                            nc.tensor.matmul(
                                px[:], lhsT[:, k, 128 * g:128 * (g + 1)],
                                wchunks[k][:, 512 * n:512 * (n + 1)],
                                start=(k == 0), stop=(k == 3))
                        xo = xpo.tile([128, 512], MM_DT, tag="xo")
                        nc.vector.tensor_add(
                            xo[:], px[:], bias_bc[:, 512 * n:512 * (n + 1)])
                        xp_dst = xp_enc_d if is_enc else xp_lm_d
                        nc.sync.dma_start(
                            xp_dst[128 * g:128 * (g + 1),
                                   512 * n:512 * (n + 1)], xo[:])

        # =========================================================
        # Recurrences
        # =========================================================
        with tc.tile_pool(name="state", bufs=2) as state, \
             tc.tile_pool(name="sig", bufs=2) as sig_p, \
             tc.tile_pool(name="tmp", bufs=2) as tmp_p, \
             tc.tile_pool(name="xp", bufs=2) as xp_p, \
             tc.tile_pool(name="psum_g", bufs=2, space="PSUM") as psum_g, \
             tc.tile_pool(name="psum_h", bufs=2, space="PSUM") as psum_h:

            ACT = mybir.ActivationFunctionType

            def gates_nonlin(pg):
                # one batched sigmoid covers i (p0-15), f (p32-47), o (p64-79);
                # f and o are then remapped to partition 0 by DVE/Pool copies
                # (cheaper than three separate ACT passes).
                sigb = sig_p.tile([80, 1024], EW_DT, tag="sigb")
                nc.scalar.activation(sigb[:], pg[0:80, :], ACT.Sigmoid)
                si = sigb[0:16, :]
                sf = sig_p.tile([16, 1024], EW_DT, tag="sf")
                nc.vector.tensor_copy(sf[:], sigb[32:48, :])
                so = sig_p.tile([16, 1024], EW_DT, tag="so")
                nc.gpsimd.tensor_copy(so[:], sigb[64:80, :])
                tg = sig_p.tile([16, 1024], EW_DT, tag="tg")
                nc.scalar.activation(tg[:], pg[96:112, :], ACT.Tanh)
                return si, sf, so, tg

            def cell_update(si, sf, so, tg, c_cur):
                t1 = tmp_p.tile([16, 1024], EW_DT, tag="t1")
                nc.vector.tensor_mul(t1[:], sf[:], c_cur[:])
                t2 = tmp_p.tile([16, 1024], EW_DT, tag="t2")
                nc.gpsimd.tensor_mul(t2[:], si, tg[:])
                c_new = state.tile([16, 1024], EW_DT, tag="c")
                nc.vector.tensor_add(c_new[:], t1[:], t2[:])
                tc_t = tmp_p.tile([16, 1024], EW_DT, tag="t1")
                nc.scalar.activation(tc_t[:], c_new[:], ACT.Tanh)
                h_new = state.tile([16, 1024], EW_DT, tag="h")
                nc.vector.tensor_mul(h_new[:], so[:], tc_t[:])
                return c_new, h_new

            def transpose_h(h_new, out_sb_ap):
                ph = psum_h.tile([128, 8, 16], dt.float32, tag="ph")
                for k in range(8):
                    nc.tensor.transpose(ph[:, k, :],
                                        h_new[:, 128 * k:128 * (k + 1)],
                                        ident[0:16, 0:16])
                nc.vector.tensor_copy(out_sb_ap, ph[:])

            # ---- Phase C: encoder (fwd + bwd fused), 64 steps ----
            whh_enc_ctx = tc.tile_pool(name="whh_enc", bufs=1)
            whh_enc_p = whh_enc_ctx.__enter__()
            whh_f_sb = whh_enc_p.tile([128, 4, 4 * He], MM_DT, tag="wenc_f")
            whh_b_sb = whh_enc_p.tile([128, 4, 4 * He], MM_DT, tag="wenc_b")
            for k in range(4):
                nc.sync.dma_start(whh_f_sb[:, k, :], whh_f_d[128 * k:128 * (k + 1), :])
                nc.sync.dma_start(whh_b_sb[:, k, :], whh_b_d[128 * k:128 * (k + 1), :])

            h_cur = state.tile([16, 1024], EW_DT, tag="h")
            c_cur = state.tile([16, 1024], EW_DT, tag="c")
            nc.sync.dma_start(h_cur[:, 0:512], h0_d[0])
            nc.sync.dma_start(h_cur[:, 512:1024], h0_d[1])
            nc.sync.dma_start(c_cur[:, 0:512], c0_d[0])
            nc.sync.dma_start(c_cur[:, 512:1024], c0_d[1])
            hT = state.tile([128, 8, 16], MM_DT, tag="hT")
            transpose_h(h_cur, hT[:])

            for s in range(LT_STEPS):
                xf = xp_p.tile([16, 2048], MM_DT, tag="xf")
                nc.sync.dma_start(xf[:], xp_enc_d[16 * s:16 * s + 16, 0:2048])
                xb = xp_p.tile([16, 2048], MM_DT, tag="xb")
                rb = 16 * (Lt - 1 - s)
                nc.sync.dma_start(xb[:], xp_enc_d[rb:rb + 16, 2048:4096])

                pg = psum_g.tile([128, 1024], dt.float32, tag="pg")
                for k in range(4):
                    for j in range(4):
                        nc.tensor.matmul(
                            pg[32 * j:32 * j + 16, 0:512], hT[:, k, :],
                            whh_f_sb[:, k, 512 * j:512 * (j + 1)],
                            start=(k == 0), stop=False,
                            tile_position=(0, 32 * j))
                for k in range(4):
                    for j in range(4):
                        nc.tensor.matmul(
                            pg[32 * j:32 * j + 16, 512:1024], hT[:, 4 + k, :],
                            whh_b_sb[:, k, 512 * j:512 * (j + 1)],
                            start=(k == 0), stop=False,
                            tile_position=(0, 32 * j))
                for j in range(4):
                    nc.tensor.matmul(
                        pg[32 * j:32 * j + 16, 0:512], ident_mm[:],
                        xf[:, 512 * j:512 * (j + 1)],
                        start=False, stop=True, tile_position=(0, 32 * j))
                for j in range(4):
                    nc.tensor.matmul(
                        pg[32 * j:32 * j + 16, 512:1024], ident_mm[:],
                        xb[:, 512 * j:512 * (j + 1)],
                        start=False, stop=True, tile_position=(0, 32 * j))

                si, sf, so, tg = gates_nonlin(pg)
                c_cur, h_cur = cell_update(si, sf, so, tg, c_cur)
                hT = state.tile([128, 8, 16], MM_DT, tag="hT")
                transpose_h(h_cur, hT[:])

            # ---- Phase D: reshape final states -> LM initial state ----
            h_lm = state.tile([16, 1024], EW_DT, tag="h")
            c_lm = state.tile([16, 1024], EW_DT, tag="c")
            # h_lm row r<8:  [src[2r, fwd], src[2r+1, fwd]]
            # h_lm row r>=8: [src[2(r-8), bwd], src[2(r-8)+1, bwd]]
            for dst, src in ((h_lm, h_cur), (c_lm, c_cur)):
                for rh in range(2):         # 0: fwd rows (r<8), 1: bwd rows
                    for ch in range(2):     # dest col half = even/odd src row
                        nc.sync.dma_start(
                            dst[8 * rh:8 * rh + 8, 512 * ch:512 * ch + 512],
                            src[ch:16:2, 512 * rh:512 * rh + 512])
            hT = state.tile([128, 8, 16], MM_DT, tag="hT")
            transpose_h(h_lm, hT[:])
            c_cur = c_lm

            whh_enc_ctx.__exit__(None, None, None)

            # ---- Phase E: LM recurrence, 128 steps ----
            whh_lm_ctx = tc.tile_pool(name="whh_lm", bufs=1)
            whh_lm_p = whh_lm_ctx.__enter__()
            whh_sb = whh_lm_p.tile([128, 8, 4 * H], MM_DT, tag="wlm")
            for k in range(8):
                nc.sync.dma_start(whh_sb[:, k, :], whh_lm_d[128 * k:128 * (k + 1), :])

            for t in range(T_STEPS):
                xt = xp_p.tile([16, 4096], MM_DT, tag="xf")
                nc.sync.dma_start(xt[:], xp_lm_d[16 * t:16 * t + 16, :])

                pg = psum_g.tile([128, 1024], dt.float32, tag="pg")
                for k in range(8):
                    for h2 in range(2):
                        for j in range(4):
                            nc.tensor.matmul(
                                pg[32 * j:32 * j + 16, 512 * h2:512 * (h2 + 1)],
                                hT[:, k, :],
                                whh_sb[:, k, 1024 * j + 512 * h2:
                                       1024 * j + 512 * (h2 + 1)],
                                start=(k == 0), stop=False,
                                tile_position=(0, 32 * j))
                for h2 in range(2):
                    for j in range(4):
                        nc.tensor.matmul(
                            pg[32 * j:32 * j + 16, 512 * h2:512 * (h2 + 1)],
                            ident_mm[:],
                            xt[:, 1024 * j + 512 * h2:1024 * j + 512 * (h2 + 1)],
                            start=False, stop=True, tile_position=(0, 32 * j))

                si, sf, so, tg = gates_nonlin(pg)
                c_cur, h_new = cell_update(si, sf, so, tg, c_cur)
                transpose_h(h_new, ysT[:, :, 16 * t:16 * t + 16])
                hT = ysT[:, :, 16 * t:16 * t + 16]
            whh_lm_ctx.__exit__(None, None, None)

        # =========================================================
        # Phase F: decode (vocab-sharded): out = ysT.T @ WdecT + bdec
        # =========================================================
        with tc.tile_pool(name="wdec", bufs=3) as wdp, \
             tc.tile_pool(name="dbias", bufs=1) as dbp, \
             tc.tile_pool(name="dout", bufs=6) as dop:
            n_nc = DEC_NC or _ceil_div(VS, 512)
            bias_dec = dbp.tile([128, VS], dt.float32)
            nc.sync.dma_start(bias_dec[:], _bcast_ap(bdec_d, VS))

            for n in range(n_nc):
                nw = min(512, VS - 512 * n)
                wn = wdp.tile([128, 8, 512], MM_DT, tag="wn")
                for k in range(8):
                    nc.sync.dma_start(
                        wn[:, k, :nw],
                        wdec_d[128 * k:128 * (k + 1), 512 * n:512 * n + nw])
                for m in range(N_TOK // 128):
                    pd = psum_mm.tile([128, 512], dt.float32, tag="mm")
                    for k in range(8):
                        nc.tensor.matmul(
                            pd[:, :nw], ysT[:, k, 128 * m:128 * (m + 1)],
                            wn[:, k, :nw], start=(k == 0), stop=(k == 7))
                    ob = dop.tile([128, 512], dt.float32, tag="ob")
                    nc.vector.tensor_add(ob[:, :nw], pd[:, :nw],
                                         bias_dec[:, 512 * n:512 * n + nw])
                    nc.sync.dma_start(
                        out_d[128 * m:128 * (m + 1), 512 * n:512 * n + nw],
                        ob[:, :nw])

    nc.compile()
    return nc


def _prep_inputs(inputs):
    f32 = np.float32
    x = np.asarray(inputs["x"]).astype(np.int64)
    table = np.asarray(inputs["table"]).astype(np.int64)
    idx_t = np.zeros((128, 24), np.int32)
    xf = x.T.reshape(-1)        # row r = 16t+b
    tf = table.T.reshape(-1)
    for g in range(16):
        idx_t[:, g] = xf[128 * g:128 * (g + 1)]
    for g in range(8):
        idx_t[:, 16 + g] = tf[128 * g:128 * (g + 1)]

    pe = _gate_perm(He)
    pl = _gate_perm(H)
    wih_enc_t = np.concatenate(
        [np.asarray(inputs["Wih_f"])[pe].T, np.asarray(inputs["Wih_b"])[pe].T],
        axis=1).astype(MM_NP)                       # [512, 4096]
    b_enc = np.concatenate(
        [np.asarray(inputs["b_f"])[pe], np.asarray(inputs["b_b"])[pe]])[None]
    wih_lm_t = np.ascontiguousarray(np.asarray(inputs["Wih_lm"])[pl].T).astype(MM_NP)
    whh_f_t = np.ascontiguousarray(np.asarray(inputs["Whh_f"])[pe].T).astype(MM_NP)
    whh_b_t = np.ascontiguousarray(np.asarray(inputs["Whh_b"])[pe].T).astype(MM_NP)
    whh_lm_t = np.ascontiguousarray(np.asarray(inputs["Whh_lm"])[pl].T).astype(MM_NP)

    wdec = np.asarray(inputs["Wdec"]).astype(f32)
    bdec = np.asarray(inputs["bdec"]).astype(f32)
    wdec_pad = np.zeros((NCORES * VS, H), f32)
    wdec_pad[:V] = wdec
    bdec_pad = np.zeros(NCORES * VS, f32)
    bdec_pad[:V] = bdec

    common = dict(
        embed=np.ascontiguousarray(np.asarray(inputs["embed"], f32)),
        tembed=np.ascontiguousarray(np.asarray(inputs["table_embed"], f32)),
        idx_t=idx_t,
        wih_enc_t=wih_enc_t, wih_lm_t=wih_lm_t,
        whh_f_t=whh_f_t, whh_b_t=whh_b_t, whh_lm_t=whh_lm_t,
        b_enc=b_enc.astype(f32),
        b_lm=np.asarray(inputs["b_lm"])[pl][None].astype(f32),
        enc_h0=np.asarray(inputs["enc_h0"], f32),
        enc_c0=np.asarray(inputs["enc_c0"], f32),
    )
    in_maps = []
    for c in range(NCORES):
        m = dict(common)
        m["wdec_t"] = np.ascontiguousarray(
            wdec_pad[c * VS:(c + 1) * VS].T).astype(MM_NP)
        m["bdec_s"] = np.ascontiguousarray(bdec_pad[None, c * VS:(c + 1) * VS])
        in_maps.append(m)
    return in_maps


def kernel(**inputs) -> np.ndarray:
    import time as _time
    if "nc" not in _CACHE:
        _CACHE["nc"] = build_bass()
    nc = _CACHE["nc"]
    in_maps = _prep_inputs(inputs)
    res = None
    for attempt in range(3):
        try:
            res = run_bass_kernel_spmd(nc, in_maps, core_ids=list(range(NCORES)))
            break
        except Exception:
            # transient NRT_EXEC_UNIT_UNRECOVERABLE has been observed right
            # after a crashed predecessor session; back off and retry
            if attempt == 2:
                raise
            _time.sleep(10)
    outs = [res.results[c]["out"] for c in range(NCORES)]
    full = np.concatenate(outs, axis=1)[:, :V]       # [2048, 50257]
    return np.ascontiguousarray(full.reshape(T, B, V))


if __name__ == "__main__":
    nc = build_bass()
    print("build ok")



# revision 46
# speedup vs baseline: 1.8911x; 1.8911x over previous
"""Trainium2 Bass kernel for nn_ConditionedLM (BiLSTM table encoder -> LSTM LM -> vocab decoder).

Strategy (8 NeuronCores, SPMD — one program, per-core data):
  * Embedding gathers + input projections (x @ Wih.T + b) computed on-device,
    replicated on every core (tables shipped bf16 to halve gather traffic).
  * LSTM recurrences (encoder fwd+bwd fused, then the LM) replicated on all
    cores: per-step cost is dominated by streaming Whh through the PE array
    (batch-independent), and an 8-way shard would need a per-step all-gather
    whose latency floor exceeds the whole step.  Replication also means the
    decode needs no communication.
  * Decoder matmul (ys @ Wdec.T + bdec) sharded over vocab: core m computes
    logits[:, :, m*VS:(m+1)*VS].

Per-step structure: gates computed with 4 PE column-group-tiled matmuls
(tile_position=(0,32j)), one per gate, concurrent in the 128x128 array
(M=B=16 per 32-col strip).  Gate g lands in PSUM partitions [32g, 32g+16).
Weights host-reordered i,f,g,o -> i,f,o,g so one Sigmoid over partitions
0..79 covers (i, f, o) and one Tanh over [96,112) covers g.  The x
projections enter the accumulation as K=16 identity matmuls emitted FIRST
(start=True), then the Whh k-chunks accumulate half-major (h2=0 fully, then
h2=1), so the nonlinear chain for cols [0,512) overlaps the PE's second
half-block.  The chain itself is chunked (CH cols) and pipelined across
Activation/Vector/Pool with no partition-realign copies (engines read
operands at their native partition bases).  h chunks are re-transposed by
the PE as they emerge and feed the NEXT step's matmuls k-chunk by k-chunk;
for the LM the transposed h goes straight into the ysT buffer that decode
uses as lhsT.
"""

import numpy as np
import ml_dtypes
from contextlib import ExitStack

import concourse.bass as bass
import concourse.mybir as mybir
import concourse.tile as tile
from concourse import bacc
from concourse.bass_utils import run_bass_kernel_spmd
from concourse.masks import make_identity

dt = mybir.dt
bf16 = ml_dtypes.bfloat16

V, E, He, H = 50257, 512, 512, 1024
B, T, Lt = 16, 128, 64
NCORES = 8
VS = (V + NCORES - 1) // NCORES  # 6283 vocab rows per core (padded)
N_TOK = B * T                    # 2048
N_TAB = B * Lt                   # 1024

MM_DT = dt.bfloat16              # matmul operand dtype (accum stays fp32)
MM_NP = bf16
EW_DT = dt.float32               # elementwise/state dtype

_CACHE = {}
LT_STEPS = Lt
T_STEPS = T
DEC_NC = None
DEC_PRE = 3                      # Wdec chunks preloaded + decoded inside LM
# nonlinearity pipeline chunks (col ranges): small first chunks minimize
# the latency to the first transposed h (the PE's step-head dependency)
CHUNKS = [(0, 256), (256, 512), (512, 768), (768, 1024)]
ACT = mybir.ActivationFunctionType


def _gate_perm(h):
    # torch gate order i,f,g,o -> i,f,o,g
    return np.concatenate([np.arange(0, h), np.arange(h, 2 * h),
                           np.arange(3 * h, 4 * h), np.arange(2 * h, 3 * h)])


def _ceil_div(a, b):
    return (a + b - 1) // b


def _bcast_ap(dram_tensor, n_free):
    """AP reading dram_tensor's single row broadcast to 128 partitions."""
    return bass.AP(dram_tensor, 0, [[0, 128], [1, n_free]])


def build_bass():
    nc = bacc.Bacc()

    embT_d = nc.dram_tensor("embT", [128, (E // 128) * N_TOK], MM_DT,
                            kind="ExternalInput")
    tembT_d = nc.dram_tensor("tembT", [128, (E // 128) * N_TAB], MM_DT,
                             kind="ExternalInput")
    wih_enc_d = nc.dram_tensor("wih_enc_t", [E, 8 * He], MM_DT, kind="ExternalInput")
    wih_lm_d = nc.dram_tensor("wih_lm_t", [E, 4 * H], MM_DT, kind="ExternalInput")
    whh_f_d = nc.dram_tensor("whh_f_t", [He, 4 * He], MM_DT, kind="ExternalInput")
    whh_b_d = nc.dram_tensor("whh_b_t", [He, 4 * He], MM_DT, kind="ExternalInput")
    whh_lm_d = nc.dram_tensor("whh_lm_t", [H, 4 * H], MM_DT, kind="ExternalInput")
    b_enc_d = nc.dram_tensor("b_enc", [1, 8 * He], dt.float32, kind="ExternalInput")
    b_lm_d = nc.dram_tensor("b_lm", [1, 4 * H], dt.float32, kind="ExternalInput")
    wdec_d = nc.dram_tensor("wdec_t", [H, VS], MM_DT, kind="ExternalInput")
    bdec_d = nc.dram_tensor("bdec_s", [1, VS], dt.float32, kind="ExternalInput")
    h0_d = nc.dram_tensor("enc_h0", [2, B, He], dt.float32, kind="ExternalInput")
    c0_d = nc.dram_tensor("enc_c0", [2, B, He], dt.float32, kind="ExternalInput")
    out_d = nc.dram_tensor("out", [N_TOK, VS], dt.float32, kind="ExternalOutput")

    with tile.TileContext(nc) as tc, ExitStack() as ctx:
        # DRAM intermediates for input projections, staged in "strip layout":
        # dim1 index 32j+b = PSUM gate-strip partition, so the per-step tile
        # loads straight into the layout the chain's add expects.  Rows
        # 32j+16..32j+31 are never written (read as don't-care).
        dram = ctx.enter_context(tc.tile_pool(name="dram", bufs=1, space="DRAM"))
        xp_lm_d = dram.tile([T, 128, 1024], MM_DT)
        xf_enc_d = dram.tile([Lt, 128, 512], MM_DT)
        xb_enc_d = dram.tile([Lt, 128, 512], MM_DT)

        const = ctx.enter_context(tc.tile_pool(name="const", bufs=1))
        ident = const.tile([128, 128], dt.float32)
        make_identity(nc, ident[:])
        ident_mm = const.tile([16, 16], MM_DT)
        make_identity(nc, ident_mm[:])

        psum_mm = ctx.enter_context(
            tc.tile_pool(name="psum_mm", bufs=2, space="PSUM"))
        ysT_p = ctx.enter_context(tc.tile_pool(name="ysT", bufs=1))
        ysT = ysT_p.tile([128, 8, N_TOK], MM_DT)
        wdec_pp = ctx.enter_context(tc.tile_pool(name="wdec_pp", bufs=1))

        # =========================================================
        # Phase A+B: gather + transpose embeddings, input projections
        # =========================================================
        with tc.tile_pool(name="gather", bufs=1) as gpool, \
             tc.tile_pool(name="bias", bufs=2) as bpool, \
             tc.tile_pool(name="wih", bufs=8) as wpool, \
             tc.tile_pool(name="xpout", bufs=3) as xpo:
            bias_bcs = []
            for bias_src in (b_enc_d, b_lm_d):
                bias_bc = bpool.tile([128, 4096], dt.float32, tag="bbc")
                nc.sync.dma_start(bias_bc[:], _bcast_ap(bias_src, 4096))
                bias_bcs.append(bias_bc)

            lm_base = xp_lm_d[:, :, :]
            f_base = xf_enc_d[:, :, :]
            b_base = xb_enc_d[:, :, :]
            # host-gathered transposed embeddings -> SBUF, then project
            embT = gpool.tile([128, E // 128, N_TOK], MM_DT)
            tembT = gpool.tile([128, E // 128, N_TAB], MM_DT)
            nc.sync.dma_start(tembT[:], tembT_d[:])
            for k in range(E // 128):
                nc.sync.dma_start(
                    embT[:, k, :], embT_d[:, N_TOK * k:N_TOK * (k + 1)])
            for i, (eT, n_rows, wih_src) in enumerate(
                    ((tembT, N_TAB, wih_enc_d), (embT, N_TOK, wih_lm_d))):
                is_enc = (i == 0)
                bias_bc = bias_bcs[0 if is_enc else 1]
                wchunks = []
                for k in range(4):
                    wc = wpool.tile([128, 4096], MM_DT, tag="wih")
                    nc.scalar.dma_start(wc[:], wih_src[128 * k:128 * (k + 1), :])
                    wchunks.append(wc)
                for g in range(n_rows // 128):
                    xog = xpo.tile([128, 4096], MM_DT, tag="xo")
                    for n in range(8):
                        px = psum_mm.tile([128, 512], dt.float32, tag="mm")
                        for k in range(4):
                            nc.tensor.matmul(
                                px[:], eT[:, k, 128 * g:128 * (g + 1)],
                                wchunks[k][:, 512 * n:512 * (n + 1)],
                                start=(k == 0), stop=(k == 3))
                        nc.vector.tensor_add(
                            xog[:, 512 * n:512 * (n + 1)], px[:],
                            bias_bc[:, 512 * n:512 * (n + 1)])
                    # batched scatter to strip layout: src partition p=16dt+b
                    # -> dst [step 8g+dt, strip-row 32j+b, units]; one DMA
                    # per gate strip j (3-dim APs balance against the SBUF
                    # source), issued on alternating queues
                    eng = nc.sync if g % 2 == 0 else nc.scalar
                    if is_enc:
                        for base, half in ((f_base, 0), (b_base, 1)):
                            for j in range(4):
                                ap = bass.AP(
                                    base.tensor,
                                    base.offset + g * 8 * 65536 + 32 * j * 512,
                                    [[65536, 8], [512, 16], [1, 512]])
                                eng.dma_start(
                                    ap, xog[:, 2048 * half + 512 * j:
                                            2048 * half + 512 * (j + 1)])
                    else:
                        for j in range(4):
                            ap = bass.AP(
                                lm_base.tensor,
                                lm_base.offset + g * 8 * 131072 + 32 * j * 1024,
                                [[131072, 8], [1024, 16], [1, 1024]])
                            eng.dma_start(ap, xog[:, 1024 * j:1024 * (j + 1)])

        # =========================================================
        # Recurrences
        # =========================================================
        with tc.tile_pool(name="state", bufs=2) as state, \
             tc.tile_pool(name="sig", bufs=2) as sig_p, \
             tc.tile_pool(name="tmp", bufs=2) as tmp_p, \
             tc.tile_pool(name="xp", bufs=3) as xp_p, \
             tc.tile_pool(name="psum_g", bufs=2, space="PSUM") as psum_g, \
             tc.tile_pool(name="psum_h", bufs=2, space="PSUM") as psum_h:

            NCH = len(CHUNKS)         # chain chunks per step

            # The HW requires equal partition bases when both inputs of a
            # tensor_tensor op live in SBUF, so operands are co-located:
            # the c state sits at partitions [32,48) (pairs with sigmoid(f)
            # at sigb[32:48]) and tanh(c) at [64,80) (pairs with sigmoid(o)).
            def chain_alloc():
                """Per-step tiles for the nonlinear chain."""
                sigb = sig_p.tile([80, 1024], EW_DT, tag="sigb")
                tg = sig_p.tile([16, 1024], EW_DT, tag="tg")
                t1 = tmp_p.tile([16, 1024], EW_DT, tag="t1")
                t2 = tmp_p.tile([16, 1024], EW_DT, tag="t2")
                tcn = tmp_p.tile([80, 1024], EW_DT, tag="tc")
                c_new = state.tile([48, 1024], EW_DT, tag="c")
                h_new = state.tile([16, 1024], EW_DT, tag="h")
                return sigb, tg, t1, t2, tcn, c_new, h_new

            def chain_alloc_gs():
                gs = tmp_p.tile([112, 1024], EW_DT, tag="gs", name="gs")
                return gs

            def chain_emit(ch, gs, halves, c_prev):
                """Emit the nonlinear chain for one step, software-pipelined
                per chunk so the first h chunk emerges with minimal latency
                and no engine FIFO head-blocks.  halves = [(psum_half, xp_ap)]
                for gate cols [0,512) and [512,1024); the xp add happens here
                (gs = pg + xp) instead of as PE identity matmuls."""
                sigb, tg, t1, t2, tcn, c_new, h_new = ch

                def srcs(c):
                    lo, hi = CHUNKS[c]
                    pg, xp = halves[0 if lo < 512 else 1]
                    off = 0 if lo < 512 else 512
                    return pg, xp, slice(lo - off, hi - off)

                sl = [slice(lo, hi) for lo, hi in CHUNKS]
                for c in range(NCH + 2):
                    if c < NCH:
                        pg, xp, ps = srcs(c)
                        s = sl[c]
                        nc.vector.tensor_add(gs[:, s], pg[0:112, ps],
                                             xp[0:112, ps])
                    if 1 <= c <= NCH:
                        s = sl[c - 1]
                        nc.scalar.activation(sigb[:, s], gs[0:80, s], ACT.Sigmoid)
                        nc.scalar.activation(tg[:, s], gs[96:112, s], ACT.Tanh)
                        nc.vector.tensor_mul(t1[:, s], sigb[32:48, s],
                                             c_prev[32:48, s])
                        nc.gpsimd.tensor_mul(t2[:, s], sigb[0:16, s], tg[:, s])
                        nc.vector.tensor_add(c_new[32:48, s], t1[:, s], t2[:, s])
                    if c >= 2:
                        s = sl[c - 2]
                        nc.scalar.activation(tcn[64:80, s], c_new[32:48, s],
                                             ACT.Tanh)
                        nc.vector.tensor_mul(h_new[:, s], sigb[64:80, s],
                                             tcn[64:80, s])

            def chain_trans(ch, ph, hT_chunk_of):
                """Transpose h chunk c -> psum -> hT slices (per 128-col k)."""
                sigb, tg, t1, t2, tcn, c_new, h_new = ch

                def do(c):
                    k0, k1 = CHUNKS[c][0] // 128, CHUNKS[c][1] // 128
                    for kk in range(k0, k1):
                        nc.tensor.transpose(ph[:, kk, :],
                                            h_new[:, 128 * kk:128 * (kk + 1)],
                                            ident[0:16, 0:16])
                    nc.vector.tensor_copy(
                        hT_chunk_of(k0, k1 - k0), ph[:, k0:k1, :])
                return do

            # ---- Phase C: encoder (fwd + bwd fused), 64 steps ----
            whh_enc_ctx = tc.tile_pool(name="whh_enc", bufs=1)
            whh_enc_p = whh_enc_ctx.__enter__()
            whh_f_sb = whh_enc_p.tile([128, 4, 4 * He], MM_DT, tag="wenc_f")
            whh_b_sb = whh_enc_p.tile([128, 4, 4 * He], MM_DT, tag="wenc_b")
            for k in range(4):
                nc.sync.dma_start(whh_f_sb[:, k, :], whh_f_d[128 * k:128 * (k + 1), :])
                nc.sync.dma_start(whh_b_sb[:, k, :], whh_b_d[128 * k:128 * (k + 1), :])

            h_cur = state.tile([16, 1024], EW_DT, tag="h")
            c_cur = state.tile([48, 1024], EW_DT, tag="c")
            nc.sync.dma_start(h_cur[:, 0:512], h0_d[0])
            nc.sync.dma_start(h_cur[:, 512:1024], h0_d[1])
            nc.sync.dma_start(c_cur[32:48, 0:512], c0_d[0])
            nc.sync.dma_start(c_cur[32:48, 512:1024], c0_d[1])
            hT = state.tile([128, 8, 16], MM_DT, tag="hT")
            ph0 = psum_h.tile([128, 8, 16], dt.float32, tag="ph")
            for k in range(8):
                nc.tensor.transpose(ph0[:, k, :],
                                    h_cur[:, 128 * k:128 * (k + 1)],
                                    ident[0:16, 0:16])
            nc.vector.tensor_copy(hT[:], ph0[:])

            def enc_xp(s):
                """DMA strip-layout x-projections for encoder step s."""
                xf = xp_p.tile([128, 512], MM_DT, tag="xf")
                nc.sync.dma_start(xf[:], xf_enc_d[s])
                xb = xp_p.tile([128, 512], MM_DT, tag="xb")
                nc.sync.dma_start(xb[:], xb_enc_d[Lt - 1 - s])
                return xf, xb

            def enc_gates_k(pg, hT_k_ap, whh_sb, k, stop):
                """One k-chunk of the encoder gates matmul into half-tile pg."""
                for j in range(4):
                    nc.tensor.matmul(
                        pg[32 * j:32 * j + 16, :],
                        hT_k_ap, whh_sb[:, k, 512 * j:512 * (j + 1)],
                        start=(k == 0), stop=stop,
                        tile_position=(0, 32 * j))

            # Half-step-offset pipeline: fwd and bwd gates use disjoint
            # hT chunks (K splits), so the PE computes one direction's gates
            # while the other direction's nonlinear chain runs.  Emission
            # order per step s:
            #   chain-fwd(s) | trans-bwd(s-1) + bwd-gates(s) |
            #   chain-bwd(s) | trans-fwd(s) + fwd-gates(s+1)
            def chain_half(ch, gs, pg, xp, half, c_prev):
                # fwd cascade on DVE, bwd cell ops on Pool, so the two
                # directions' chains don't serialize through one engine
                # FIFO (the gs add reads PSUM, so it must stay on DVE)
                sigb, tg, t1, t2, tcn, c_new, h_new = ch
                if half == 0:
                    e_t1, e_t2, e_addc, e_h = (nc.vector, nc.gpsimd,
                                               nc.vector, nc.vector)
                else:
                    e_t1, e_t2, e_addc, e_h = (nc.gpsimd, nc.vector,
                                               nc.gpsimd, nc.gpsimd)
                cs = [c for c in range(NCH)
                      if (CHUNKS[c][0] < 512) == (half == 0)]
                off = 0 if half == 0 else 512
                for c in cs:
                    lo, hi = CHUNKS[c]
                    s_ = slice(lo, hi)
                    ps = slice(lo - off, hi - off)
                    nc.vector.tensor_add(gs[:, s_], pg[0:112, ps],
                                         xp[0:112, ps])
                for c in cs:
                    s_ = slice(*CHUNKS[c])
                    nc.scalar.activation(sigb[:, s_], gs[0:80, s_], ACT.Sigmoid)
                    nc.scalar.activation(tg[:, s_], gs[96:112, s_], ACT.Tanh)
                    e_t1.tensor_mul(t1[:, s_], sigb[32:48, s_],
                                    c_prev[32:48, s_])
                    e_t2.tensor_mul(t2[:, s_], sigb[0:16, s_], tg[:, s_])
                    e_addc.tensor_add(c_new[32:48, s_], t1[:, s_], t2[:, s_])
                for c in cs:
                    s_ = slice(*CHUNKS[c])
                    nc.scalar.activation(tcn[64:80, s_], c_new[32:48, s_],
                                         ACT.Tanh)
                    e_h.tensor_mul(h_new[:, s_], sigb[64:80, s_],
                                   tcn[64:80, s_])

            # bootstrap: fwd gates of step 0 from hT(init)
            pga = psum_g.tile([128, 512], dt.float32, tag="pga")
            xfb = enc_xp(0)
            for k in range(4):
                enc_gates_k(pga, hT[:, k, :], whh_f_sb, k, k == 3)

            hT_prev, ph_prev, trans_prev = hT, None, None
            for s in range(LT_STEPS):
                last = (s == LT_STEPS - 1)
                ch = chain_alloc()
                gs = chain_alloc_gs()
                # 1) fwd chain of step s (pga(s) complete)
                chain_half(ch, gs, pga, xfb[0], 0, c_cur)
                # 2) bwd gates of step s, consuming hT-bwd(s-1) as the
                #    previous step's bwd transposes land
                pgb = psum_g.tile([128, 512], dt.float32, tag="pgb")
                if s == 0:
                    for k in range(4):
                        enc_gates_k(pgb, hT_prev[:, 4 + k, :], whh_b_sb, k,
                                    k == 3)
                else:
                    for c in range(NCH):
                        if CHUNKS[c][0] < 512:
                            continue
                        trans_prev(c)
                        for kk in range(CHUNKS[c][0] // 128,
                                        CHUNKS[c][1] // 128):
                            enc_gates_k(pgb, hT_prev[:, kk, :], whh_b_sb,
                                        kk - 4, kk == 7)
                # 3) bwd chain of step s
                chain_half(ch, gs, pgb, xfb[1], 1, c_cur)
                # 4) fwd gates of step s+1, consuming hT-fwd(s)
                if not last:
                    xfb = enc_xp(s + 1)
                    hT = state.tile([128, 8, 16], MM_DT, tag="hT")
                    ph = psum_h.tile([128, 8, 16], dt.float32, tag="ph")
                    trans = chain_trans(ch, ph, lambda k0, n: hT[:, k0:k0 + n, :])
                    pga = psum_g.tile([128, 512], dt.float32, tag="pga")
                    for c in range(NCH):
                        if CHUNKS[c][0] >= 512:
                            continue
                        trans(c)
                        for kk in range(CHUNKS[c][0] // 128,
                                        CHUNKS[c][1] // 128):
                            enc_gates_k(pga, hT[:, kk, :], whh_f_sb, kk,
                                        kk == 3)
                    hT_prev, ph_prev, trans_prev = hT, ph, trans
                c_cur = ch[5]
                h_cur = ch[6]

            # ---- Phase D: reshape final states -> LM initial state ----
            h_lm = state.tile([16, 1024], EW_DT, tag="h")
            c_lm = state.tile([48, 1024], EW_DT, tag="c")
            # h_lm row r<8:  [src[2r, fwd], src[2r+1, fwd]]
            # h_lm row r>=8: [src[2(r-8), bwd], src[2(r-8)+1, bwd]]
            for dst, src, p0 in ((h_lm, h_cur, 0), (c_lm, c_cur, 32)):
                for rh in range(2):         # 0: fwd rows (r<8), 1: bwd rows
                    for chh in range(2):    # dest col half = even/odd src row
                        nc.sync.dma_start(
                            dst[p0 + 8 * rh:p0 + 8 * rh + 8,
                                512 * chh:512 * chh + 512],
                            src[p0 + chh:p0 + 16:2,
                                512 * rh:512 * rh + 512])
            hT = state.tile([128, 8, 16], MM_DT, tag="hT")
            ph0 = psum_h.tile([128, 8, 16], dt.float32, tag="ph")
            for k in range(8):
                nc.tensor.transpose(ph0[:, k, :],
                                    h_lm[:, 128 * k:128 * (k + 1)],
                                    ident[0:16, 0:16])
            nc.vector.tensor_copy(hT[:], ph0[:])
            c_cur = c_lm

            whh_enc_ctx.__exit__(None, None, None)

            # ---- Phase E: LM recurrence, 128 steps ----
            whh_lm_ctx = tc.tile_pool(name="whh_lm", bufs=1)
            whh_lm_p = whh_lm_ctx.__enter__()
            whh_sb = whh_lm_p.tile([128, 8, 4 * H], MM_DT, tag="wlm")
            for k in range(8):
                nc.sync.dma_start(whh_sb[:, k, :], whh_lm_d[128 * k:128 * (k + 1), :])
            # preload DEC_PRE vocab chunks of Wdec; their (n, m) decode units
            # are interleaved into LM-step PE idle gaps
            wn_pre = wdec_pp.tile([128, DEC_PRE, 8, 512], MM_DT)
            for n in range(DEC_PRE):
                for k in range(8):
                    nc.scalar.dma_start(
                        wn_pre[:, n, k, :],
                        wdec_d[128 * k:128 * (k + 1), 512 * n:512 * (n + 1)])
            bias_pre = wdec_pp.tile([128, DEC_PRE * 512], MM_DT)
            nc.gpsimd.dma_start(bias_pre[:], _bcast_ap(bdec_d, DEC_PRE * 512))
            dec_units = []           # (n, m) units decoded during the LM
            dec_done = set()

            def emit_dec_unit(n, m):
                pd = psum_mm.tile([128, 512], dt.float32, tag="mm")
                for k in range(8):
                    nc.tensor.matmul(
                        pd[:], ysT[:, k, 128 * m:128 * (m + 1)],
                        wn_pre[:, n, k, :], start=(k == 0), stop=(k == 7))
                ob = xp_p.tile([128, 512], dt.float32, tag="ob")
                nc.vector.tensor_add(ob[:], pd[:],
                                     bias_pre[:, 512 * n:512 * (n + 1)])
                nc.sync.dma_start(
                    out_d[128 * m:128 * (m + 1), 512 * n:512 * (n + 1)],
                    ob[:])
                dec_done.add((n, m))

            def lm_xp(t):
                xt = xp_p.tile([128, 1024], MM_DT, tag="xf")
                nc.sync.dma_start(xt[:], xp_lm_d[t])
                return xt

            def lm_gates_k(pg, hT_k_ap, k, h2, stop):
                for j in range(4):
                    nc.tensor.matmul(
                        pg[32 * j:32 * j + 16, :],
                        hT_k_ap,
                        whh_sb[:, k, 1024 * j + 512 * h2:
                               1024 * j + 512 * (h2 + 1)],
                        start=(k == 0), stop=stop, tile_position=(0, 32 * j))

            # bootstrap: step 0 gates from hT(init); h2-major
            pga = psum_g.tile([128, 512], dt.float32, tag="pga")
            pgb = psum_g.tile([128, 512], dt.float32, tag="pgb")
            xt_c = lm_xp(0)
            for h2, pg in ((0, pga), (1, pgb)):
                for k in range(8):
                    lm_gates_k(pg, hT[:, k, :], k, h2, k == 7)

            for t in range(T_STEPS):
                last = (t == T_STEPS - 1)
                ch = chain_alloc()
                sigb, tg, t1, t2, tcn, c_new, h_new = ch
                gs = chain_alloc_gs()
                ph = psum_h.tile([128, 8, 16], dt.float32, tag="ph")
                trans = chain_trans(
                    ch, ph,
                    lambda k0, n, t=t: ysT[:, k0:k0 + n, 16 * t:16 * t + 16])
                if not last:
                    pga_n = psum_g.tile([128, 512], dt.float32, tag="pga")
                    pgb_n = psum_g.tile([128, 512], dt.float32, tag="pgb")
                    xt_n = lm_xp(t + 1)
                if t % 2 == 0 and dec_units:
                    emit_dec_unit(*dec_units.pop(0))
                if t % 8 == 7 and t < T_STEPS - 1:
                    m_ready = t // 8
                    dec_units.extend((n, m_ready) for n in range(DEC_PRE))
                chain_emit(ch, gs,
                           [(pga, xt_c[:, 0:512]), (pgb, xt_c[:, 512:1024])],
                           c_cur)
                # h2=0 of next step's gates interleaved with transposes,
                # h2=1 afterwards (all hT chunks then ready)
                for c in range(NCH):
                    trans(c)
                    if not last:
                        for kk in range(CHUNKS[c][0] // 128,
                                        CHUNKS[c][1] // 128):
                            lm_gates_k(pga_n,
                                       ysT[:, kk, 16 * t:16 * t + 16],
                                       kk, 0, kk == 7)
                if not last:
                    for kk in range(8):
                        lm_gates_k(pgb_n, ysT[:, kk, 16 * t:16 * t + 16],
                                   kk, 1, kk == 7)
                c_cur = c_new
                if not last:
                    pga, pgb = pga_n, pgb_n
                    xt_c = xt_n
            whh_lm_ctx.__exit__(None, None, None)

        # =========================================================
        # Phase F: decode (vocab-sharded): out = ysT.T @ WdecT + bdec
        # =========================================================
        with tc.tile_pool(name="wdec", bufs=3) as wdp, \
             tc.tile_pool(name="dbias", bufs=1) as dbp, \
             tc.tile_pool(name="dout", bufs=6) as dop:
            n_nc = DEC_NC or _ceil_div(VS, 512)
            bias_dec = dbp.tile([128, VS], dt.float32)
            nc.sync.dma_start(bias_dec[:], _bcast_ap(bdec_d, VS))

            for n in range(n_nc):
                nw = min(512, VS - 512 * n)
                if n < DEC_PRE:
                    wn = wn_pre[:, n, :, :]
                else:
                    wnt = wdp.tile([128, 8, 512], MM_DT, tag="wn")
                    for k in range(8):
                        nc.sync.dma_start(
                            wnt[:, k, :nw],
                            wdec_d[128 * k:128 * (k + 1), 512 * n:512 * n + nw])
                    wn = wnt[:, :, :]
                for m in range(N_TOK // 128):
                    if (n, m) in dec_done:
                        continue
                    pd = psum_mm.tile([128, 512], dt.float32, tag="mm")
                    for k in range(8):
                        nc.tensor.matmul(
                            pd[:, :nw], ysT[:, k, 128 * m:128 * (m + 1)],
                            wn[:, k, :nw], start=(k == 0), stop=(k == 7))
                    ob = dop.tile([128, 512], dt.float32, tag="ob")
                    nc.vector.tensor_add(ob[:, :nw], pd[:, :nw],
                                         bias_dec[:, 512 * n:512 * n + nw])
                    nc.sync.dma_start(
                        out_d[128 * m:128 * (m + 1), 512 * n:512 * n + nw],
                        ob[:, :nw])

    nc.compile()
    return nc


def _embT_host(tbl, idx):
    """Gather embedding rows for flat token order r=16t+b and lay out as
    [128 partitions, (E//128) * n_rows] (the transposed lhsT layout)."""
    g = np.asarray(tbl, np.float32)[idx]            # [n, E]
    n = g.shape[0]
    gt = g.T.reshape(E // 128, 128, n)              # [k, p, n]
    return np.ascontiguousarray(
        gt.transpose(1, 0, 2).reshape(128, -1)).astype(MM_NP)


def _prep_inputs(inputs):
    f32 = np.float32
    x = np.asarray(inputs["x"]).astype(np.int64)
    table = np.asarray(inputs["table"]).astype(np.int64)
    xf = x.T.reshape(-1)        # row r = 16t+b
    tf = table.T.reshape(-1)

    pe = _gate_perm(He)
    pl = _gate_perm(H)
    wih_enc_t = np.concatenate(
        [np.asarray(inputs["Wih_f"])[pe].T, np.asarray(inputs["Wih_b"])[pe].T],
        axis=1).astype(MM_NP)                       # [512, 4096]
    b_enc = np.concatenate(
        [np.asarray(inputs["b_f"])[pe], np.asarray(inputs["b_b"])[pe]])[None]
    wih_lm_t = np.ascontiguousarray(np.asarray(inputs["Wih_lm"])[pl].T).astype(MM_NP)
    whh_f_t = np.ascontiguousarray(np.asarray(inputs["Whh_f"])[pe].T).astype(MM_NP)
    whh_b_t = np.ascontiguousarray(np.asarray(inputs["Whh_b"])[pe].T).astype(MM_NP)
    whh_lm_t = np.ascontiguousarray(np.asarray(inputs["Whh_lm"])[pl].T).astype(MM_NP)

    wdec = np.asarray(inputs["Wdec"]).astype(f32)
    bdec = np.asarray(inputs["bdec"]).astype(f32)
    wdec_pad = np.zeros((NCORES * VS, H), f32)
    wdec_pad[:V] = wdec
    bdec_pad = np.zeros(NCORES * VS, f32)
    bdec_pad[:V] = bdec

    common = dict(
        embT=_embT_host(inputs["embed"], xf),
        tembT=_embT_host(inputs["table_embed"], tf),
        wih_enc_t=wih_enc_t, wih_lm_t=wih_lm_t,
        whh_f_t=whh_f_t, whh_b_t=whh_b_t, whh_lm_t=whh_lm_t,
        b_enc=b_enc.astype(f32),
        b_lm=np.asarray(inputs["b_lm"])[pl][None].astype(f32),
        enc_h0=np.asarray(inputs["enc_h0"], f32),
        enc_c0=np.asarray(inputs["enc_c0"], f32),
    )
    in_maps = []
    for c in range(NCORES):
        m = dict(common)
        m["wdec_t"] = np.ascontiguousarray(
            wdec_pad[c * VS:(c + 1) * VS].T).astype(MM_NP)
        m["bdec_s"] = np.ascontiguousarray(bdec_pad[None, c * VS:(c + 1) * VS])
        in_maps.append(m)
    return in_maps


def kernel(**inputs) -> np.ndarray:
    import time as _time
    if "nc" not in _CACHE:
        _CACHE["nc"] = build_bass()
    nc = _CACHE["nc"]
    in_maps = _prep_inputs(inputs)
    res = None
    for attempt in range(3):
        try:
            res = run_bass_kernel_spmd(nc, in_maps, core_ids=list(range(NCORES)))
            break
        except Exception:
            # transient NRT_EXEC_UNIT_UNRECOVERABLE has been observed right
            # after a crashed predecessor session; back off and retry
            if attempt == 2:
                raise
            _time.sleep(10)
    outs = [res.results[c]["out"] for c in range(NCORES)]
    full = np.concatenate(outs, axis=1)[:, :V]       # [2048, 50257]
    return np.ascontiguousarray(full.reshape(T, B, V))


if __name__ == "__main__":
    nc = build_bass()
    print("build ok")
